# revision 1
# baseline (speedup 1.0000x reference)
"""RGCN 2-layer + pair-MLP Trainium2 kernel (8 NeuronCores, SPMD).

Strategy: the output only depends on node embeddings at nest/food nodes
(T, ~2k), so layer 2 aggregates only in-edges of T (~33k edges) and
layer 1 only computes h1 at S1 = T + sources of those edges (~29k nodes,
~470k in-edges).  Everything runs in fp16 (tolerance 2e-2): per-edge
messages are fetched with dma_gather across 4 parallel SWDGE queues,
segment-summed into PSUM via one-hot matmuls (one 128-wide one-hot per
128-edge window, relation-pure windows -> 4 PSUM regions of 128 dst
columns, root term folded in as pseudo-relation R), then transformed
with the per-relation weights.  h1[S1] is AllGathered in fp16 between
layers; the final pair MLP computes per-core partials and AllReduces.
"""
import sys
sys.path.insert(0, '/opt/trn_rl_repo')

import numpy as np
import concourse.bass as bass
import concourse.bacc as bacc
import concourse.tile as tile
import concourse.mybir as mybir
import concourse.bass_isa as bass_isa
from concourse.bass_utils import run_bass_kernel_spmd
from concourse.masks import make_identity

F32 = mybir.dt.float32
F16 = mybir.dt.float16
I32 = mybir.dt.int32
I16 = mybir.dt.int16

NC = 8          # cores
GMAX = 1024     # max gather call length (slots); >1024 wedges the SWDGE ucode
GRP = 5         # dst tiles per group (PSUM residency: 5+2+1 = 8 banks)
NQUEUES = 4     # parallel SWDGE descriptor-generation queues
SB = 32         # one-hot S matrices built per DVE instruction


class Stream:
    """Core-invariant padded edge-stream layout for one layer.

    Edges (+root pseudo-edges, rel=R) sorted by (core, grp, chunk, dd,
    rel, src).  Cell (grp, m, dd, r) sizes are max-over-cores so the
    layout is identical on every core; per-(grp, m) segments are padded
    to x16 (idx column granularity).  Gather calls split segments at
    GMAX; matmul windows are call-local 128-slot tiles; items map
    (window x overlapped cell) -> one one-hot matmul into PSUM region
    (dd, r).
    """

    def __init__(self, nrr, dt, chs, nchunk, e_src, e_dpos, e_rel, e_rec,
                 e_core):
        # e_dpos: core-local dst index (0..ND-1 within core)
        self.RR = nrr
        self.DT = dt
        ng = -(-dt // GRP)
        self.NG = ng
        dd = e_dpos >> 7
        grp = dd // GRP
        m = e_src // chs
        order = np.lexsort((e_src, e_rel, dd, m, grp, e_core))
        s_src = e_src[order]
        s_dloc = e_dpos[order]
        s_dd = dd[order]
        s_rel = e_rel[order]
        s_rec = e_rec[order]
        s_core = e_core[order]
        s_m = m[order]
        s_grp = grp[order]

        ncell_pc = ng * nchunk * GRP * nrr
        cellk = (((s_grp * nchunk + s_m) * GRP + (s_dd - s_grp * GRP)) * nrr
                 + s_rel)
        cnt = np.bincount(s_core * ncell_pc + cellk,
                          minlength=NC * ncell_pc)
        P = cnt.reshape(NC, ng, nchunk, GRP, nrr).max(axis=0)  # [NG,CH,G,RR]
        seglen = P.sum(axis=(2, 3))                            # [NG, CH]
        seglen_pad = ((seglen + 15) // 16) * 16
        seg_off = np.zeros((ng, nchunk), np.int64)
        seg_off.flat[1:] = np.cumsum(seglen_pad.ravel())[:-1]
        self.tot_slots = int(seglen_pad.sum())

        # cell offsets within segment (ddl-major, r-minor == sort order)
        cell_off = np.zeros((ng, nchunk, GRP, nrr), np.int64)
        pf = P.reshape(ng * nchunk, GRP * nrr)
        co = np.zeros_like(pf)
        co[:, 1:] = np.cumsum(pf, axis=1)[:, :-1]
        cell_off[...] = co.reshape(ng, nchunk, GRP, nrr)
        cell_glob = seg_off[:, :, None, None] + cell_off       # global slot

        cell_start = np.zeros(NC * ncell_pc + 1, np.int64)
        cell_start[1:] = np.cumsum(cnt)
        rank = np.arange(len(s_src), dtype=np.int64) - \
            cell_start[s_core * ncell_pc + cellk]
        gidx = np.unravel_index(cellk, (ng, nchunk, GRP, nrr))
        slot = cell_glob[gidx] + rank

        # idx table (int16 chunk-local src), 16-wrapped
        ncols = self.tot_slots // 16
        idx16 = np.zeros((NC, 16, ncols), np.int16)
        idx16[s_core, slot % 16, slot // 16] = (s_src - s_m * chs).astype(
            np.int16)
        self.idx128 = np.tile(idx16, (1, 8, 1))                # [NC,128,ncols]
        self.ncols = ncols

        seg_pad = np.full((NC, self.tot_slots), -1.0, np.float16)
        rec_pad = np.zeros((NC, self.tot_slots), np.float16)
        seg_pad[s_core, slot] = (s_dloc & 127).astype(np.float16)
        rec_pad[s_core, slot] = s_rec.astype(np.float16)

        # gather calls + items
        self.calls = []    # per grp: list of (go, m, col0, L, ntl)
        self.items = []    # per grp: list of (go, t, dd, r, first, last, it)
        iseg_cols = []
        irec_cols = []
        first_seen = {}
        last_of = {}
        go = 0
        all_calls = []
        all_items = []
        for g in range(ng):
            gcalls = []
            gitems = []
            for mm in range(nchunk):
                L = int(seglen_pad[g, mm])
                off = 0
                while off < L:
                    gl = min(GMAX, L - off)
                    base = int(seg_off[g, mm]) + off
                    ntl = (gl + 127) // 128
                    gcalls.append((go, mm, base // 16, gl, ntl))
                    for t in range(ntl):
                        w0 = base + t * 128
                        w1 = min(base + gl, w0 + 128)
                        # overlapped cells
                        for ddl in range(GRP):
                            dd_ = g * GRP + ddl
                            if dd_ >= dt:
                                continue
                            for r in range(nrr):
                                c0 = int(cell_glob[g, mm, ddl, r])
                                c1 = c0 + int(P[g, mm, ddl, r])
                                a, b = max(w0, c0), min(w1, c1)
                                if a >= b:
                                    continue
                                it = len(iseg_cols)
                                sc = np.full((NC, 128), -1.0, np.float16)
                                rc = np.zeros((NC, 128), np.float16)
                                sc[:, a - w0:b - w0] = seg_pad[:, a:b]
                                rc[:, a - w0:b - w0] = rec_pad[:, a:b]
                                iseg_cols.append(sc)
                                irec_cols.append(rc)
                                first_seen[(dd_, r)] = True
                                gitems.append([go, t, dd_, r, False, False,
                                               it])
                    off += gl
                    go += 1
            all_calls.append(gcalls)
            all_items.append(gitems)
        # regions with no items would leave PSUM garbage: every (dd, r)
        # must be touched at least once.  Add a dummy all-masked item on
        # an existing window for any empty region.
        for g in range(ng):
            for ddl in range(GRP):
                dd_ = g * GRP + ddl
                if dd_ >= dt:
                    continue
                for r in range(nrr):
                    if (dd_, r) in first_seen:
                        continue
                    go0, mm0, c00, L0, _ = all_calls[g][0]
                    it = len(iseg_cols)
                    iseg_cols.append(np.full((NC, 128), -1.0, np.float16))
                    irec_cols.append(np.zeros((NC, 128), np.float16))
                    first_seen[(dd_, r)] = True
                    all_items[g].append([go0, 0, dd_, r, False, False, it])
        # renumber item ids so each group's items are consecutive in the
        # iseg/irec tables (needed for batched one-hot builds)
        iseg2 = []
        irec2 = []
        for g in range(ng):
            for rec in all_items[g]:
                iseg2.append(iseg_cols[rec[6]])
                irec2.append(irec_cols[rec[6]])
                rec[6] = len(iseg2) - 1
        iseg_cols = iseg2
        irec_cols = irec2
        # start/stop are per PSUM tile (2KB zero region == whole dd tile):
        # first item touching dd starts the group, last one stops it.
        for g in range(ng):
            fst = {}
            lst = {}
            for i, rec in enumerate(all_items[g]):
                dd_ = rec[2]
                if dd_ not in fst:
                    fst[dd_] = i
                lst[dd_] = i
            for dd_, i in fst.items():
                all_items[g][i][4] = True
            for dd_, i in lst.items():
                all_items[g][i][5] = True
        self.calls = all_calls
        self.items = all_items
        # per-group idx column range [c0, c1) and item id range [i0, i1)
        self.gcols = []
        self.gitems = []
        for g in range(ng):
            c0 = int(seg_off[g, 0]) // 16
            c1 = (int(seg_off[g + 1, 0]) // 16 if g + 1 < ng
                  else self.tot_slots // 16)
            self.gcols.append((c0, c1))
            its = [rec[6] for rec in all_items[g]]
            self.gitems.append((min(its), max(its) + 1))
        self.NIT = len(iseg_cols)
        self.iseg = np.stack(iseg_cols, axis=2)   # [NC, 128, NIT]
        self.irec = np.stack(irec_cols, axis=2)
        self.nchunk = nchunk
        self.chs = chs


def preprocess(inputs):
    src = np.asarray(inputs['edge_src']).astype(np.int64)
    dst = np.asarray(inputs['edge_dst']).astype(np.int64)
    rel = np.asarray(inputs['edge_type']).astype(np.int64)
    nest = np.asarray(inputs['nest']).astype(np.int64)
    food = np.asarray(inputs['food']).astype(np.int64)
    N = inputs['x'].shape[0]
    R = inputs['W_rel1'].shape[0]
    RR = R + 1

    T = np.unique(np.concatenate([nest, food]))
    inT = np.zeros(N, bool)
    inT[T] = True
    m2 = inT[dst]
    S1 = np.union1d(T, np.unique(src[m2]))
    nS1 = len(S1)
    pos1 = np.full(N, -1, np.int64)
    pos1[S1] = np.arange(nS1)
    inS1 = np.zeros(N, bool)
    inS1[S1] = True
    m1 = inS1[dst]

    ND1 = -(-nS1 // NC)
    DT1 = -(-ND1 // 128)
    NDP1 = DT1 * 128
    # chunk-major padded-global id of S1 member i in h1_full:
    # chunks of CR=GRP*128 dst rows are AllGathered separately, so
    # h1_full = concat over chunks of concat over cores.
    CR = GRP * 128
    core_of = np.arange(nS1) // ND1
    dloc_of = np.arange(nS1) - core_of * ND1
    g_of = dloc_of // CR
    rows_of = np.minimum(CR, NDP1 - g_of * CR)
    gp_of = NC * CR * g_of + core_of * rows_of + (dloc_of - g_of * CR)

    # ---- layer 1 stream (dst in S1, srcs global, +root pseudo) ----
    CHS1 = 25000
    CH1 = -(-N // CHS1)
    e_src = src[m1]
    e_d = pos1[dst[m1]]
    e_rel = rel[m1]
    cnt1 = np.bincount(rel[m1] * N + dst[m1], minlength=R * N)
    rec1 = (1.0 / np.maximum(cnt1, 1)).astype(np.float32)
    e_rec = rec1[e_rel * N + dst[m1]]
    a_src = np.concatenate([e_src, S1])
    a_d = np.concatenate([e_d, np.arange(nS1)])
    a_rel = np.concatenate([e_rel, np.full(nS1, R, np.int64)])
    a_rec = np.concatenate([e_rec, np.ones(nS1, np.float32)])
    a_core = a_d // ND1
    a_dloc = a_d - a_core * ND1
    st1 = Stream(RR, DT1, CHS1, CH1, a_src, a_dloc, a_rel, a_rec, a_core)

    # ---- T sharding (inherited from S1 shard) ----
    posT_in_S1 = pos1[T]
    coreT = posT_in_S1 // ND1
    NT_c = np.bincount(coreT, minlength=NC)
    DT2 = max(1, -(-int(NT_c.max()) // 128))
    NDP2 = DT2 * 128
    tloc = np.zeros(len(T), np.int64)
    for c in range(NC):
        tloc[coreT == c] = np.arange(int(NT_c[c]))
    posT = np.full(N, -1, np.int64)
    posT[T] = tloc
    coreT_of = np.full(N, -1, np.int64)
    coreT_of[T] = coreT

    # ---- layer 2 stream (dst in T, srcs = gp ids in h1_full) ----
    NT2 = NC * NDP1
    # chunk the L2 gather table by AllGather chunk so early L2 gathers
    # only depend on the AG chunks they read
    CHS2 = NC * CR
    CH2 = -(-NT2 // CHS2)
    f_src = gp_of[pos1[src[m2]]]
    f_d = posT[dst[m2]]
    f_core = coreT_of[dst[m2]]
    f_rel = rel[m2]
    cnt2 = np.bincount(rel[m2] * N + dst[m2], minlength=R * N)
    rec2 = (1.0 / np.maximum(cnt2, 1)).astype(np.float32)
    f_rec = rec2[f_rel * N + dst[m2]]
    b_src = np.concatenate([f_src, gp_of[posT_in_S1]])
    b_d = np.concatenate([f_d, tloc])
    b_rel = np.concatenate([f_rel, np.full(len(T), R, np.int64)])
    b_rec = np.concatenate([f_rec, np.ones(len(T), np.float32)])
    b_core = np.concatenate([f_core, coreT])
    st2 = Stream(RR, DT2, CHS2, CH2, b_src, b_d, b_rel, b_rec, b_core)

    # ---- final MLP index prep ----
    B = len(nest)
    nest_core = coreT_of[nest]
    nest_tl = posT[nest]
    food_core = coreT_of[food]
    food_tl = posT[food]
    nestw = np.zeros((NC, 128, B // 16), np.int16)
    foodw = np.zeros((NC, 128, B // 16), np.int16)
    s = np.arange(B)
    for c in range(NC):
        nc_idx = np.where(nest_core == c, nest_tl, NDP2).astype(np.int16)
        fc_idx = np.where(food_core == c, food_tl, NDP2).astype(np.int16)
        w16 = np.zeros((16, B // 16), np.int16)
        w16[s % 16, s // 16] = nc_idx
        nestw[c] = np.tile(w16, (8, 1))
        w16 = np.zeros((16, B // 16), np.int16)
        w16[s % 16, s // 16] = fc_idx
        foodw[c] = np.tile(w16, (8, 1))

    # per-(dd, r, dst) aggregate scaling tables (mean recips),
    # replicated across all 128 partitions
    recd1 = np.ones((NC, DT1 * RR * 128), np.float16)
    for c in range(NC):
        i0, i1 = c * ND1, min((c + 1) * ND1, nS1)
        if i1 <= i0:
            continue
        nodes = S1[i0:i1]
        dloc = np.arange(i1 - i0)
        col0 = (dloc >> 7) * (RR * 128) + (dloc & 127)
        for r in range(R):
            recd1[c, col0 + r * 128] = rec1[r * N + nodes]
    recd1 = np.broadcast_to(recd1[:, None, :],
                            (NC, 128, DT1 * RR * 128)).copy()
    recd2 = np.ones((NC, DT2 * RR * 128), np.float16)
    for c in range(NC):
        tnodes = T[coreT == c]
        dloc = np.arange(len(tnodes))
        col0 = (dloc >> 7) * (RR * 128) + (dloc & 127)
        for r in range(R):
            recd2[c, col0 + r * 128] = rec2[r * N + tnodes]
    recd2 = np.broadcast_to(recd2[:, None, :],
                            (NC, 128, DT2 * RR * 128)).copy()

    dims = dict(N=N, R=R, RR=RR, nS1=nS1, ND1=ND1, DT1=DT1, NDP1=NDP1,
                DT2=DT2, NDP2=NDP2, NT2=NT2, CHS1=CHS1, CH1=CH1,
                CHS2=CHS2, CH2=CH2, B=B)
    return st1, st2, dims, nestw, foodw, recd1, recd2


def build(st1, st2, dims, F, H, EMB):
    nc_ = bacc.Bacc("TRN2", target_bir_lowering=False, debug=False,
                    num_devices=NC, num_swdge_queues=NQUEUES,
                    dynamic_dma_scratch_size=16384)
    RR = dims['RR']
    NDP1, NDP2, NT2, B = (dims['NDP1'], dims['NDP2'], dims['NT2'],
                          dims['B'])

    t_x16 = nc_.dram_tensor("x16", [dims['N'], F], F16,
                            kind="ExternalInput")
    t_idx1 = nc_.dram_tensor("idx1", [128, st1.ncols], I16,
                             kind="ExternalInput")
    t_is1 = nc_.dram_tensor("is1", [128, st1.NIT], F16,
                            kind="ExternalInput")
    t_recd1 = nc_.dram_tensor("recd1", [128, st1.DT * RR * 128], F16,
                              kind="ExternalInput")
    t_idx2 = nc_.dram_tensor("idx2", [128, st2.ncols], I16,
                             kind="ExternalInput")
    t_is2 = nc_.dram_tensor("is2", [128, st2.NIT], F16,
                            kind="ExternalInput")
    t_recd2 = nc_.dram_tensor("recd2", [128, st2.DT * RR * 128], F16,
                              kind="ExternalInput")
    t_w1 = nc_.dram_tensor("w1", [F, RR * H], F16, kind="ExternalInput")
    t_b1 = nc_.dram_tensor("b1", [1, H], F16, kind="ExternalInput")
    t_w2 = nc_.dram_tensor("w2", [H, RR * EMB], F16, kind="ExternalInput")
    t_b2 = nc_.dram_tensor("b2", [1, EMB], F16, kind="ExternalInput")
    t_fca = nc_.dram_tensor("fca", [EMB, H], F32, kind="ExternalInput")
    t_fcb = nc_.dram_tensor("fcb", [EMB, H], F32, kind="ExternalInput")
    t_fbias = nc_.dram_tensor("fbias", [1, H], F32, kind="ExternalInput")
    t_nest = nc_.dram_tensor("nestw", [128, B // 16], I16,
                             kind="ExternalInput")
    t_food = nc_.dram_tensor("foodw", [128, B // 16], I16,
                             kind="ExternalInput")
    t_out = nc_.dram_tensor("out", [B, H], F32, kind="ExternalOutput")

    h1_part = nc_.dram_tensor("h1_part", [NDP1, H], F16, kind="Internal")
    h1_full = nc_.dram_tensor("h1_full", [NC * NDP1, H], F16,
                              kind="Internal", addr_space="Shared")
    nd_part = nc_.dram_tensor("nd_part", [NDP2 + 1, EMB], F32,
                              kind="Internal")
    cc_fin = nc_.dram_tensor("cc_fin", [B, H], F16, kind="Internal")
    cc_fin_o = nc_.dram_tensor("cc_fin_o", [B, H], F16, kind="Internal",
                               addr_space="Shared")

    with tile.TileContext(nc_) as tc:
        with tc.tile_pool(name="const", bufs=1) as cpool, \
             tc.tile_pool(name="big", bufs=1) as bigp, \
             tc.tile_pool(name="gidx", bufs=3) as gp_idx, \
             tc.tile_pool(name="gis", bufs=6) as gp_is, \
             tc.tile_pool(name="msg", bufs=32) as msgp, \
             tc.tile_pool(name="s", bufs=4) as sp, \
             tc.tile_pool(name="ag", bufs=4) as agp, \
             tc.tile_pool(name="work", bufs=4) as wp, \
             tc.tile_pool(name="pa", bufs=GRP, space="PSUM") as pap, \
             tc.tile_pool(name="pb", bufs=2, space="PSUM") as pbp, \
             tc.tile_pool(name="pc", bufs=1, space="PSUM") as pcp:

            c_i = cpool.tile([128, 128], I32)
            nc_.gpsimd.iota(c_i[:], pattern=[[1, 128]], base=0,
                            channel_multiplier=0)
            cseg = cpool.tile([128, 128], F16)
            nc_.vector.tensor_copy(cseg[:], c_i[:])
            ctile = cpool.tile([128, SB * 128], F16)
            for j in range(SB):
                nc_.vector.tensor_copy(ctile[:, j * 128:(j + 1) * 128],
                                       cseg[:])
            ones1 = cpool.tile([1, 128], F16)
            nc_.vector.memset(ones1[:], 1.0)
            ones1f = cpool.tile([1, 128], F32)
            nc_.vector.memset(ones1f[:], 1.0)
            ident = cpool.tile([128, 128], F32)
            make_identity(nc_, ident[:])



            recd1_sb = cpool.tile([128, st1.DT * RR * 128], F16)
            nc_.sync.dma_start(out=recd1_sb[:], in_=t_recd1[:])
            recd2_sb = cpool.tile([128, st2.DT * RR * 128], F16)
            nc_.sync.dma_start(out=recd2_sb[:], in_=t_recd2[:])
            w1_sb = cpool.tile([F, RR * H], F16)
            nc_.sync.dma_start(out=w1_sb[:], in_=t_w1[:])
            b1_sb = cpool.tile([1, H], F16)
            nc_.sync.dma_start(out=b1_sb[:], in_=t_b1[:])
            w2_sb = cpool.tile([H, RR * EMB], F16)
            nc_.sync.dma_start(out=w2_sb[:], in_=t_w2[:])
            b2_sb = cpool.tile([1, EMB], F16)
            nc_.sync.dma_start(out=b2_sb[:], in_=t_b2[:])
            fca_sb = cpool.tile([EMB, H], F32)
            nc_.sync.dma_start(out=fca_sb[:], in_=t_fca[:])
            fcb_sb = cpool.tile([EMB, H], F32)
            nc_.sync.dma_start(out=fcb_sb[:], in_=t_fcb[:])
            fbias_sb = cpool.tile([1, H], F32)
            nc_.sync.dma_start(out=fbias_sb[:], in_=t_fbias[:])
            nest_sb = cpool.tile([128, B // 16], I16)
            nc_.sync.dma_start(out=nest_sb[:], in_=t_nest[:])
            food_sb = cpool.tile([128, B // 16], I16)
            nc_.sync.dma_start(out=food_sb[:], in_=t_food[:])

            def layer(st, table, chs, t_idx, t_is, recd_sb, FF, w_sb,
                      b_sb, HH, out_dram, relu, out_dt=F16,
                      after_group=None):
                # FF: feature width of gathered rows (contraction dim)
                nrows = table.shape[0]
                LAG = 2
                for g in range(st.NG):
                    if after_group is not None and g - LAG >= 0:
                        after_group(g - LAG)
                    gc0, gc1 = st.gcols[g]
                    gi0, gi1 = st.gitems[g]
                    idx_sb = gp_idx.tile([128, gc1 - gc0], I16, tag="gidx")
                    nc_.sync.dma_start(out=idx_sb[:],
                                       in_=t_idx[:, gc0:gc1])
                    is_sb = gp_is.tile([128, gi1 - gi0], F16, tag="gis")
                    nc_.sync.dma_start(out=is_sb[:], in_=t_is[:, gi0:gi1])
                    msgs = {}
                    for (go, mm, col0, L, ntl) in st.calls[g]:
                        msg = msgp.tile([128, ntl * FF], F16, tag="msg")
                        if L % 128:
                            nc_.vector.memset(msg[:, (ntl - 1) * FF:], 0.0)
                        lo = mm * chs
                        hi = min(nrows, lo + chs)
                        nc_.gpsimd.dma_gather(
                            out_ap=msg[:].rearrange("p (c e) -> p c e",
                                                    e=FF),
                            in_ap=table[lo:hi, :],
                            idxs_ap=idx_sb[:, col0 - gc0:
                                           col0 - gc0 + (L + 15) // 16],
                            num_idxs=L, num_idxs_reg=L, elem_size=FF)
                        msgs[go] = msg
                    psumA = {}
                    items = st.items[g]
                    sbatch = {}
                    for b0 in range(0, len(items), SB):
                        bk = min(SB, len(items) - b0)
                        i0 = items[b0][6] - gi0
                        assert all(items[b0 + j][6] - gi0 == i0 + j
                                   for j in range(bk))
                        S = sp.tile([128, SB * 128], F16, tag="S")
                        bis = is_sb[:, i0:i0 + bk].unsqueeze(2).broadcast_to(
                            (128, bk, 128))
                        sv = S[:, :bk * 128].rearrange("p (k e) -> p k e",
                                                       e=128)
                        nc_.vector.tensor_tensor(
                            out=sv, in0=bis,
                            in1=ctile[:, :bk * 128].rearrange(
                                "p (k e) -> p k e", e=128),
                            op=mybir.AluOpType.is_equal)
                        for j in range(bk):
                            sbatch[b0 + j] = (S, j)
                    for ii, (go, t, dd, r, first, last, it) in                             enumerate(items):
                        if dd not in psumA:
                            pa_tile = pap.tile([128, RR * 128], F32,
                                               tag="A", space="PSUM")
                            psumA[dd] = pa_tile
                        S, j = sbatch[ii]
                        nc_.tensor.matmul(
                            out=psumA[dd][:, r * 128:(r + 1) * 128],
                            lhsT=msgs[go][:, t * FF:(t + 1) * FF],
                            rhs=S[:, j * 128:(j + 1) * 128],
                            start=bool(first), stop=bool(last))
                    for dd in sorted(psumA.keys()):
                        aggT = agp.tile([128, RR * 128], F16, tag="aggT")
                        nc_.scalar.activation(
                            out=aggT[:], in_=psumA[dd][:],
                            func=mybir.ActivationFunctionType.Copy)
                        nc_.vector.tensor_tensor(
                            out=aggT[:], in0=aggT[:],
                            in1=recd_sb[:, dd * RR * 128:
                                        (dd + 1) * RR * 128],
                            op=mybir.AluOpType.mult)
                        psumB = pbp.tile([128, HH], F32, tag="B",
                                         space="PSUM")
                        for r in range(RR):
                            nc_.tensor.matmul(
                                out=psumB[:],
                                lhsT=aggT[:, r * 128:(r + 1) * 128],
                                rhs=w_sb[:, r * HH:(r + 1) * HH],
                                start=(r == 0), stop=False)
                        nc_.tensor.matmul(
                            out=psumB[:], lhsT=ones1[:1, :],
                            rhs=b_sb[:1, :], start=False, stop=True)
                        o_sb = wp.tile([128, HH], out_dt, tag="osb")
                        nc_.scalar.activation(
                            out=o_sb[:], in_=psumB[:],
                            func=(mybir.ActivationFunctionType.Relu if relu
                                  else mybir.ActivationFunctionType.Copy))
                        nc_.sync.dma_start(
                            out=out_dram[dd * 128:(dd + 1) * 128, :],
                            in_=o_sb[:])
                if after_group is not None:
                    for g in range(max(0, st.NG - LAG), st.NG):
                        after_group(g)

            # layer 1 with per-group chunked AllGather (chunk-major
            # h1_full layout: base_g + core*rows_g + (dloc - 640*g))
            CR = GRP * 128

            def ag_chunk(g):
                lo = g * CR
                rows = min(CR, NDP1 - lo)
                base = NC * lo
                nc_.gpsimd.collective_compute(
                    "AllGather", mybir.AluOpType.bypass,
                    replica_groups=[list(range(NC))],
                    ins=[h1_part[lo:lo + rows]],
                    outs=[h1_full[base:base + NC * rows]])

            layer(st1, t_x16, dims['CHS1'], t_idx1, t_is1, recd1_sb, F,
                  w1_sb, b1_sb, H, h1_part, relu=True,
                  after_group=ag_chunk)
            # layer 2
            layer(st2, h1_full, dims['CHS2'], t_idx2, t_is2, recd2_sb, H,
                  w2_sb, b2_sb, EMB, nd_part, relu=False, out_dt=F32)
            # zero sentinel row
            zrow = wp.tile([1, EMB], F32, tag="zrow")
            nc_.vector.memset(zrow[:], 0.0)
            nc_.sync.dma_start(out=nd_part[NDP2:NDP2 + 1, :],
                               in_=zrow[:1, :])
            # final pair MLP: one dma_gather each for all nest/food rows
            nfall = cpool.tile([128, (B // 128) * EMB], F32)
            ffall = cpool.tile([128, (B // 128) * EMB], F32)
            HB = B // 2
            for h in range(2):
                nc_.gpsimd.dma_gather(
                    out_ap=nfall[:, h * (HB // 128) * EMB:
                                 (h + 1) * (HB // 128) * EMB].rearrange(
                                     "p (c e) -> p c e", e=EMB),
                    in_ap=nd_part[:, :],
                    idxs_ap=nest_sb[:, h * (HB // 16):(h + 1) * (HB // 16)],
                    num_idxs=HB, num_idxs_reg=HB, elem_size=EMB)
                nc_.gpsimd.dma_gather(
                    out_ap=ffall[:, h * (HB // 128) * EMB:
                                 (h + 1) * (HB // 128) * EMB].rearrange(
                                     "p (c e) -> p c e", e=EMB),
                    in_ap=nd_part[:, :],
                    idxs_ap=food_sb[:, h * (HB // 16):(h + 1) * (HB // 16)],
                    num_idxs=HB, num_idxs_reg=HB, elem_size=EMB)
            for pt in range(B // 128):
                psumC = pcp.tile([128, 128], F32, tag="C", space="PSUM")
                nc_.tensor.transpose(
                    out=psumC[:EMB, :],
                    in_=nfall[:, pt * EMB:(pt + 1) * EMB],
                    identity=ident[:])
                nfT = wp.tile([EMB, 128], F32, tag="nfT")
                nc_.vector.tensor_copy(nfT[:], psumC[:EMB, :])
                psumC2 = pcp.tile([128, 128], F32, tag="C", space="PSUM")
                nc_.tensor.transpose(
                    out=psumC2[:EMB, :],
                    in_=ffall[:, pt * EMB:(pt + 1) * EMB],
                    identity=ident[:])
                ffT = wp.tile([EMB, 128], F32, tag="ffT")
                nc_.vector.tensor_copy(ffT[:], psumC2[:EMB, :])
                psumD = pbp.tile([128, H], F32, tag="B", space="PSUM")
                nc_.tensor.matmul(out=psumD[:], lhsT=nfT[:], rhs=fca_sb[:],
                                  start=True, stop=False)
                nc_.tensor.matmul(out=psumD[:], lhsT=ffT[:], rhs=fcb_sb[:],
                                  start=False, stop=False)
                nc_.tensor.matmul(out=psumD[:], lhsT=ones1f[:1, :],
                                  rhs=fbias_sb[:1, :], start=False,
                                  stop=True)
                fo = wp.tile([128, H], F16, tag="fo")
                nc_.vector.tensor_copy(fo[:], psumD[:])
                nc_.sync.dma_start(out=cc_fin[pt * 128:(pt + 1) * 128, :],
                                   in_=fo[:])
            nc_.gpsimd.collective_compute(
                "AllReduce", mybir.AluOpType.add,
                replica_groups=[list(range(NC))],
                ins=[cc_fin[:]], outs=[cc_fin_o[:]])
            for pt in range(B // 128):
                ft = wp.tile([128, H], F16, tag="ft")
                nc_.sync.dma_start(
                    out=ft[:], in_=cc_fin_o[pt * 128:(pt + 1) * 128, :])
                fo2 = wp.tile([128, H], F32, tag="fo2")
                nc_.scalar.activation(out=fo2[:], in_=ft[:],
                                      func=mybir.ActivationFunctionType.Tanh)
                nc_.sync.dma_start(out=t_out[pt * 128:(pt + 1) * 128, :],
                                   in_=fo2[:])
    # Spread Pool-engine DMAs across the 4 SWDGE queues.  Each DMASW
    # completion-sem lane must stay locked to one queue (per-queue FIFO
    # completion order); the tile layer already assigned lanes, so read
    # them back and set queue = lane % 4.
    from concourse.tile_scheduler import PROC_NAMES
    for bb in nc_.main_func.blocks:
        for inst in bb.instructions:
            proc = getattr(inst, 'bass_scheduled_proc', None)
            if proc is None:
                continue
            pname = PROC_NAMES[proc]
            if not pname.startswith("DMASW"):
                continue
            q = int(pname[5:]) % NQUEUES
            if isinstance(inst, mybir.InstDMAGatherAnt):
                inst.queue_num = q
            elif isinstance(inst, mybir.InstDMACopy):
                qname = str(getattr(inst, 'queue', '') or '')
                if qname.startswith("qPoolDynamic"):
                    inst.queue = "qPoolDynamic" + (str(q) if q else "")
    nc_.compile()
    return nc_


def kernel(x, edge_src, edge_dst, edge_type, edge_attr, nest, food,
           W_rel1, W_root1, b1, W_rel2, W_root2, b2, fc_W, fc_b,
           _runner=None):
    x = np.asarray(x, np.float32)
    N, F = x.shape
    H = np.asarray(W_root1).shape[1]
    EMB = np.asarray(W_root2).shape[1]
    inputs = dict(edge_src=edge_src, edge_dst=edge_dst,
                  edge_type=edge_type, nest=nest, food=food, x=x,
                  W_rel1=W_rel1)
    st1, st2, dims, nestw, foodw, recd1, recd2 = preprocess(inputs)
    nc_ = build(st1, st2, dims, F, H, EMB)

    RR = dims['RR']
    W_rel1 = np.asarray(W_rel1, np.float32)
    W_rel2 = np.asarray(W_rel2, np.float32)
    w1 = np.concatenate([W_rel1.transpose(1, 0, 2).reshape(F, -1),
                         np.asarray(W_root1, np.float32)],
                        axis=1).astype(np.float16)
    w2 = np.concatenate([W_rel2.transpose(1, 0, 2).reshape(H, -1),
                         np.asarray(W_root2, np.float32)],
                        axis=1).astype(np.float16)
    fc_W = np.asarray(fc_W, np.float32)
    x16 = x.astype(np.float16)

    in_maps = []
    for c in range(NC):
        in_maps.append(dict(
            x16=x16,
            idx1=st1.idx128[c], is1=st1.iseg[c], recd1=recd1[c],
            idx2=st2.idx128[c], is2=st2.iseg[c], recd2=recd2[c],
            w1=w1, b1=np.asarray(b1, np.float16).reshape(1, -1),
            w2=w2, b2=np.asarray(b2, np.float16).reshape(1, -1),
            fca=fc_W[:EMB].astype(np.float32),
            fcb=fc_W[EMB:].astype(np.float32),
            fbias=(np.asarray(fc_b, np.float32).reshape(1, -1) /
                   NC).astype(np.float32),
            nestw=nestw[c], foodw=foodw[c],
        ))
    runner = _runner or (lambda n, im: run_bass_kernel_spmd(
        n, im, core_ids=list(range(NC))))
    res = runner(nc_, in_maps)
    return res.results[0]["out"]



# revision 13
# speedup vs baseline: 1.3489x; 1.3489x over previous
"""RGCN 2-layer + pair-MLP Trainium2 kernel (8 NeuronCores, SPMD).

Strategy: the output only depends on node embeddings at nest/food nodes
(T ~2k), so layer 2 aggregates only in-edges of T (~33k edges) and
layer 1 only computes h1 at S1 = T + sources of those edges (~29k nodes,
~470k in-edges).  Everything runs in fp16 (tolerance 2e-2): per-edge
messages are fetched with dma_gather across 4 parallel SWDGE queues,
segment-summed into PSUM via one-hot matmuls (one 128-wide one-hot per
item), then transformed with the per-relation weights (root term folded
in as pseudo-relation R, mean recips applied via a per-(dd,rel,dst)
broadcast table built on-chip from a [1,cols] row).

v2 vs v1:
  * no h1 AllGather: layer-2 edges are sharded by the core that OWNS the
    source's h1 row; each core computes partial aggregates for ALL T
    nodes (global layout), transforms them, and a single small AllReduce
    of nd [2048,128] fp16 replaces the 7.6MB chunked AllGather.  The
    final pair-MLP is then computed redundantly on every core (no
    masking, no sentinel row, no output AllReduce).
  * padding cells pooled over relations + degree-balanced snake
    assignment of S1 nodes to (core, dst-tile) bins: ~17% fewer gather
    slots.
  * recd mean-recip tables shipped as [1, cols] rows and broadcast to
    128 partitions on-chip with K=1 matmuls (removes a 3.8MB startup
    DMA).
  * final MLP uses transpose-mode gathers (features land on partitions)
    so the identity-transpose matmuls disappear; b2 is folded into the
    final bias host-side.
"""
import sys
sys.path.insert(0, '/opt/trn_rl_repo')

import numpy as np
import concourse.bass as bass
import concourse.bacc as bacc
import concourse.tile as tile
import concourse.mybir as mybir
import concourse.bass_isa as bass_isa
from concourse.bass_utils import run_bass_kernel_spmd

F32 = mybir.dt.float32
F16 = mybir.dt.float16
I32 = mybir.dt.int32
I16 = mybir.dt.int16

NC = 8          # cores
GMAX = 1024     # max gather call length (slots); >1024 wedges the SWDGE ucode
GRP = 5         # L1 dst tiles per group (PSUM residency: 5+2+1 = 8 banks)
GRP2 = 4        # L2 dst tiles per group
NQUEUES = 4     # parallel SWDGE descriptor-generation queues
SB = 32         # one-hot S matrices built per DVE instruction


class Stream:
    """Core-invariant padded edge-stream layout for one layer.

    Edges (+root pseudo-edges, rel=R) sorted by (core, grp, chunk, dd,
    rel, src).  Cell (grp, m, dd) sizes (POOLED over rel) are
    max-over-cores so the layout is identical on every core;
    per-(grp, m) segments are padded to x16 (idx column granularity).
    Gather calls split segments at GMAX; matmul windows are call-local
    128-slot tiles; items map (window x overlapped cell x rel) -> one
    one-hot matmul into PSUM region (dd, r).
    """

    def __init__(self, nrr, dt, grp, chs, nchunk, e_src, e_dst, e_rel,
                 e_core):
        # e_dst: dst index within this stream's padded dst space [0,dt*128)
        self.RR = nrr
        self.DT = dt
        self.GRP = grp
        ng = -(-dt // grp)
        self.NG = ng
        dd = e_dst >> 7
        g = dd // grp
        m = e_src // chs
        order = np.lexsort((e_src, e_rel, dd, m, g, e_core))
        s_src = e_src[order]
        s_dst = e_dst[order]
        s_dd = dd[order]
        s_rel = e_rel[order]
        s_core = e_core[order]
        s_m = m[order]
        s_g = g[order]

        ncell_pc = ng * nchunk * grp
        cellk = (s_g * nchunk + s_m) * grp + (s_dd - s_g * grp)
        cnt = np.bincount(s_core * ncell_pc + cellk,
                          minlength=NC * ncell_pc)
        P = cnt.reshape(NC, ng, nchunk, grp).max(axis=0)  # [NG, CH, G]
        seglen = P.sum(axis=2)                            # [NG, CH]
        seglen_pad = ((seglen + 15) // 16) * 16
        seg_off = np.zeros((ng, nchunk), np.int64)
        seg_off.flat[1:] = np.cumsum(seglen_pad.ravel())[:-1]
        self.tot_slots = int(seglen_pad.sum())

        # cell offsets within segment (ddl order == sort order)
        pf = P.reshape(ng * nchunk, grp)
        co = np.zeros_like(pf)
        co[:, 1:] = np.cumsum(pf, axis=1)[:, :-1]
        cell_off = co.reshape(ng, nchunk, grp)
        cell_glob = seg_off[:, :, None] + cell_off        # global slot

        cell_start = np.zeros(NC * ncell_pc + 1, np.int64)
        cell_start[1:] = np.cumsum(cnt)
        rank = np.arange(len(s_src), dtype=np.int64) - \
            cell_start[s_core * ncell_pc + cellk]
        gidx = np.unravel_index(cellk, (ng, nchunk, grp))
        slot = cell_glob[gidx] + rank

        # idx table (int16 chunk-local src), 16-wrapped
        ncols = self.tot_slots // 16
        idx16 = np.zeros((NC, 16, ncols), np.int16)
        idx16[s_core, slot % 16, slot // 16] = (s_src - s_m * chs).astype(
            np.int16)
        self.idx128 = np.tile(idx16, (1, 8, 1))           # [NC,128,ncols]
        self.ncols = ncols

        seg_pad = np.full((NC, self.tot_slots), -1.0, np.float16)
        rel_pad = np.full((NC, self.tot_slots), -1, np.int8)
        seg_pad[s_core, slot] = (s_dst & 127).astype(np.float16)
        rel_pad[s_core, slot] = s_rel.astype(np.int8)

        # gather calls + items
        iseg_cols = []
        first_seen = {}
        go = 0
        all_calls = []
        all_items = []
        for gg in range(ng):
            gcalls = []
            gitems = []
            for mm in range(nchunk):
                L = int(seglen_pad[gg, mm])
                off = 0
                while off < L:
                    gl = min(GMAX, L - off)
                    base = int(seg_off[gg, mm]) + off
                    ntl = (gl + 127) // 128
                    gcalls.append((go, mm, base // 16, gl, ntl))
                    for t in range(ntl):
                        w0 = base + t * 128
                        w1 = min(base + gl, w0 + 128)
                        for ddl in range(grp):
                            dd_ = gg * grp + ddl
                            if dd_ >= dt:
                                continue
                            c0 = int(cell_glob[gg, mm, ddl])
                            c1 = c0 + int(P[gg, mm, ddl])
                            a, b = max(w0, c0), min(w1, c1)
                            if a >= b:
                                continue
                            relseg = rel_pad[:, a:b]
                            for r in range(nrr):
                                mask = relseg == r
                                if not mask.any():
                                    continue
                                it = len(iseg_cols)
                                sc = np.full((NC, 128), -1.0, np.float16)
                                sc[:, a - w0:b - w0] = np.where(
                                    mask, seg_pad[:, a:b], -1.0)
                                iseg_cols.append(sc)
                                first_seen[(dd_, r)] = True
                                gitems.append([go, t, dd_, r, False, False,
                                               it])
                    off += gl
                    go += 1
            all_calls.append(gcalls)
            all_items.append(gitems)
        # regions with no items would trip PSUM read-before-write: every
        # (dd, r) must be touched at least once.  Add a dummy all-masked
        # item on an existing window for any empty region.
        for gg in range(ng):
            for ddl in range(grp):
                dd_ = gg * grp + ddl
                if dd_ >= dt:
                    continue
                for r in range(nrr):
                    if (dd_, r) in first_seen:
                        continue
                    go0, mm0, c00, L0, _ = all_calls[gg][0]
                    it = len(iseg_cols)
                    iseg_cols.append(np.full((NC, 128), -1.0, np.float16))
                    first_seen[(dd_, r)] = True
                    all_items[gg].append([go0, 0, dd_, r, False, False, it])
        # renumber item ids so each group's items are consecutive in the
        # iseg table (needed for batched one-hot builds)
        iseg2 = []
        for gg in range(ng):
            for rec in all_items[gg]:
                iseg2.append(iseg_cols[rec[6]])
                rec[6] = len(iseg2) - 1
        iseg_cols = iseg2
        # start/stop are per PSUM tile (2KB zero region == whole dd tile):
        # first item touching dd starts the group, last one stops it.
        for gg in range(ng):
            fst = {}
            lst = {}
            for i, rec in enumerate(all_items[gg]):
                dd_ = rec[2]
                if dd_ not in fst:
                    fst[dd_] = i
                lst[dd_] = i
            for dd_, i in fst.items():
                all_items[gg][i][4] = True
            for dd_, i in lst.items():
                all_items[gg][i][5] = True
        self.calls = all_calls
        self.items = all_items
        # per-group idx column range [c0, c1) and item id range [i0, i1)
        self.gcols = []
        self.gitems = []
        for gg in range(ng):
            c0 = int(seg_off[gg, 0]) // 16
            c1 = (int(seg_off[gg + 1, 0]) // 16 if gg + 1 < ng
                  else self.tot_slots // 16)
            self.gcols.append((c0, c1))
            its = [rec[6] for rec in all_items[gg]]
            self.gitems.append((min(its), max(its) + 1))
        self.NIT = len(iseg_cols)
        self.iseg = np.stack(iseg_cols, axis=2)   # [NC, 128, NIT]
        self.nchunk = nchunk
        self.chs = chs


def preprocess(inputs):
    src = np.asarray(inputs['edge_src']).astype(np.int64)
    dst = np.asarray(inputs['edge_dst']).astype(np.int64)
    rel = np.asarray(inputs['edge_type']).astype(np.int64)
    nest = np.asarray(inputs['nest']).astype(np.int64)
    food = np.asarray(inputs['food']).astype(np.int64)
    N = inputs['x'].shape[0]
    R = inputs['W_rel1'].shape[0]
    RR = R + 1

    T = np.unique(np.concatenate([nest, food]))
    nT = len(T)
    posTg = np.full(N, -1, np.int64)
    posTg[T] = np.arange(nT)
    inT = np.zeros(N, bool)
    inT[T] = True
    m2 = inT[dst]
    S1 = np.union1d(T, np.unique(src[m2]))
    nS1 = len(S1)
    inS1 = np.zeros(N, bool)
    inS1[S1] = True
    m1 = inS1[dst]

    # ---- degree-balanced snake assignment of S1 -> (core, dd, col) ----
    DT1 = -(-nS1 // (128 * NC))
    NDP1 = DT1 * 128
    nbins = NC * DT1
    deg = np.bincount(dst[m1], minlength=N)[S1] + 1   # +1 root pseudo-edge
    ordn = np.argsort(-deg, kind='stable')            # S1-relative
    nfull = -(-nS1 // nbins)                          # 126
    binseq = np.arange(nbins)
    snake = np.concatenate([binseq if (i % 2 == 0) else binseq[::-1]
                            for i in range(nfull)])[:nS1]
    colseq = np.repeat(np.arange(nfull), nbins)[:nS1]
    bin_of = np.empty(nS1, np.int64)
    col_of = np.empty(nS1, np.int64)
    bin_of[ordn] = snake
    col_of[ordn] = colseq
    core1 = bin_of // DT1
    dd1 = bin_of % DT1
    slot1 = dd1 * 128 + col_of                        # local slot on core
    core1_of = np.full(N, -1, np.int64)
    slot1_of = np.full(N, -1, np.int64)
    core1_of[S1] = core1
    slot1_of[S1] = slot1

    # ---- layer 1 stream (dst in S1, srcs global, +root pseudo) ----
    CHS1 = 25000
    CH1 = -(-N // CHS1)
    e_src = src[m1]
    e_dst = slot1_of[dst[m1]]
    e_rel = rel[m1]
    e_core = core1_of[dst[m1]]
    cnt1 = np.bincount(rel[m1] * N + dst[m1], minlength=R * N)
    rec1 = (1.0 / np.maximum(cnt1, 1)).astype(np.float32)
    a_src = np.concatenate([e_src, S1])
    a_dst = np.concatenate([e_dst, slot1])
    a_rel = np.concatenate([e_rel, np.full(nS1, R, np.int64)])
    a_core = np.concatenate([e_core, core1])
    st1 = Stream(RR, DT1, GRP, CHS1, CH1, a_src, a_dst, a_rel, a_core)

    # ---- layer 2 stream: dst = global T slot, src = local h1 slot on
    # the core owning the source's h1 row (edge-parallel by src) ----
    DT2 = -(-nT // 128)
    NT = DT2 * 128
    CHS2 = GRP * 128                                  # = one L1 group
    CH2 = -(-NDP1 // CHS2)
    f_src = slot1_of[src[m2]]
    f_core = core1_of[src[m2]]
    f_dst = posTg[dst[m2]]
    f_rel = rel[m2]
    cnt2 = np.bincount(rel[m2] * N + dst[m2], minlength=R * N)
    rec2 = (1.0 / np.maximum(cnt2, 1)).astype(np.float32)
    b_src = np.concatenate([f_src, slot1_of[T]])
    b_dst = np.concatenate([f_dst, np.arange(nT)])
    b_rel = np.concatenate([f_rel, np.full(nT, R, np.int64)])
    b_core = np.concatenate([f_core, core1_of[T]])
    st2 = Stream(RR, DT2, GRP2, CHS2, CH2, b_src, b_dst, b_rel, b_core)

    # ---- recd rows: [1, DT*RR*128] mean recips (root block = 1) ----
    recd1 = np.ones((NC, DT1 * RR * 128), np.float16)
    node_at = np.full((NC, DT1 * 128), -1, np.int64)
    node_at[core1, slot1] = S1
    for c in range(NC):
        nodes = node_at[c]
        valid = nodes >= 0
        dloc = np.arange(DT1 * 128)
        col0 = (dloc >> 7) * (RR * 128) + (dloc & 127)
        for r in range(R):
            v = np.ones(DT1 * 128, np.float32)
            v[valid] = rec1[r * N + nodes[valid]]
            recd1[c, col0 + r * 128] = v
    recd2 = np.ones((1, DT2 * RR * 128), np.float16)
    dloc = np.arange(nT)
    col0 = (dloc >> 7) * (RR * 128) + (dloc & 127)
    for r in range(R):
        recd2[0, col0 + r * 128] = rec2[r * N + T]

    # ---- final MLP index prep (global T rows; same for all cores) ----
    B = len(nest)
    s = np.arange(B)
    nestw = np.zeros((16, B // 16), np.int16)
    nestw[s % 16, s // 16] = posTg[nest].astype(np.int16)
    nestw = np.tile(nestw, (8, 1))
    foodw = np.zeros((16, B // 16), np.int16)
    foodw[s % 16, s // 16] = posTg[food].astype(np.int16)
    foodw = np.tile(foodw, (8, 1))

    dims = dict(N=N, R=R, RR=RR, nS1=nS1, DT1=DT1, NDP1=NDP1,
                DT2=DT2, NT=NT, CHS1=CHS1, CH1=CH1, CHS2=CHS2, CH2=CH2,
                B=B)
    return st1, st2, dims, nestw, foodw, recd1, recd2


def build(st1, st2, dims, F, H, EMB):
    nc_ = bacc.Bacc("TRN2", target_bir_lowering=False, debug=False,
                    num_devices=NC, num_swdge_queues=NQUEUES,
                    dynamic_dma_scratch_size=16384)
    RR = dims['RR']
    NDP1, NT, B = dims['NDP1'], dims['NT'], dims['B']
    DT1, DT2 = dims['DT1'], dims['DT2']

    t_x16 = nc_.dram_tensor("x16", [dims['N'], F], F16,
                            kind="ExternalInput")
    t_idx1 = nc_.dram_tensor("idx1", [128, st1.ncols], I16,
                             kind="ExternalInput")
    t_is1 = nc_.dram_tensor("is1", [128, st1.NIT], F16,
                            kind="ExternalInput")
    t_recd1 = nc_.dram_tensor("recd1", [1, DT1 * RR * 128], F16,
                              kind="ExternalInput")
    t_idx2 = nc_.dram_tensor("idx2", [128, st2.ncols], I16,
                             kind="ExternalInput")
    t_is2 = nc_.dram_tensor("is2", [128, st2.NIT], F16,
                            kind="ExternalInput")
    t_recd2 = nc_.dram_tensor("recd2", [1, DT2 * RR * 128], F16,
                              kind="ExternalInput")
    t_w1 = nc_.dram_tensor("w1", [F, RR * H], F16, kind="ExternalInput")
    t_b1 = nc_.dram_tensor("b1", [1, H], F16, kind="ExternalInput")
    t_w2 = nc_.dram_tensor("w2", [H, RR * EMB], F16, kind="ExternalInput")
    t_fca = nc_.dram_tensor("fca", [128, H], F16, kind="ExternalInput")
    t_fcb = nc_.dram_tensor("fcb", [128, H], F16, kind="ExternalInput")
    t_fbias = nc_.dram_tensor("fbias", [1, H], F16, kind="ExternalInput")
    t_nest = nc_.dram_tensor("nestw", [128, B // 16], I16,
                             kind="ExternalInput")
    t_food = nc_.dram_tensor("foodw", [128, B // 16], I16,
                             kind="ExternalInput")
    t_out = nc_.dram_tensor("out", [B, H], F32, kind="ExternalOutput")

    h1_part = nc_.dram_tensor("h1_part", [NDP1, H], F16, kind="Internal")
    nd_part = nc_.dram_tensor("nd_part", [NT, 128], F16, kind="Internal")
    nd_full = nc_.dram_tensor("nd_full", [NT, 128], F16, kind="Internal",
                              addr_space="Shared")

    n_l2_calls = sum(len(c) for c in st2.calls)

    with tile.TileContext(nc_) as tc:
        with tc.tile_pool(name="const", bufs=1) as cpool, \
             tc.tile_pool(name="gidx", bufs=3) as gp_idx, \
             tc.tile_pool(name="msg", bufs=28) as msgp, \
             tc.tile_pool(name="msg2", bufs=n_l2_calls + 1) as msg2p, \
             tc.tile_pool(name="s", bufs=3) as sp, \
             tc.tile_pool(name="ag", bufs=4) as agp, \
             tc.tile_pool(name="rrow", bufs=3) as rrp, \
             tc.tile_pool(name="recd", bufs=6) as rp, \
             tc.tile_pool(name="work", bufs=4) as wp, \
             tc.tile_pool(name="pa", bufs=GRP, space="PSUM") as pap, \
             tc.tile_pool(name="pb", bufs=2, space="PSUM") as pbp, \
             tc.tile_pool(name="pc", bufs=1, space="PSUM") as pcp:

            c_i = cpool.tile([128, 128], I32)
            nc_.gpsimd.iota(c_i[:], pattern=[[1, 128]], base=0,
                            channel_multiplier=0)
            cseg = cpool.tile([128, 128], F16)
            nc_.vector.tensor_copy(cseg[:], c_i[:])
            ones1 = cpool.tile([1, 128], F16)
            nc_.vector.memset(ones1[:], 1.0)

            w1_sb = cpool.tile([F, RR * H], F16)
            nc_.sync.dma_start(out=w1_sb[:], in_=t_w1[:])
            b1_sb = cpool.tile([1, H], F16)
            nc_.sync.dma_start(out=b1_sb[:], in_=t_b1[:])
            w2_sb = cpool.tile([H, RR * EMB], F16)
            nc_.sync.dma_start(out=w2_sb[:], in_=t_w2[:])
            fca_sb = cpool.tile([128, H], F16)
            nc_.sync.dma_start(out=fca_sb[:], in_=t_fca[:])
            fcb_sb = cpool.tile([128, H], F16)
            nc_.sync.dma_start(out=fcb_sb[:], in_=t_fcb[:])
            fbias_sb = cpool.tile([1, H], F16)
            nc_.sync.dma_start(out=fbias_sb[:], in_=t_fbias[:])
            nest_sb = cpool.tile([128, B // 16], I16)
            nc_.sync.dma_start(out=nest_sb[:], in_=t_nest[:])
            food_sb = cpool.tile([128, B // 16], I16)
            nc_.sync.dma_start(out=food_sb[:], in_=t_food[:])
            is1_sb = cpool.tile([128, st1.NIT], F16)
            nc_.sync.dma_start(out=is1_sb[:], in_=t_is1[:])
            idx2_sb = cpool.tile([128, st2.ncols], I16)
            nc_.sync.dma_start(out=idx2_sb[:], in_=t_idx2[:])
            is2_sb = cpool.tile([128, st2.NIT], F16)
            nc_.sync.dma_start(out=is2_sb[:], in_=t_is2[:])

            def make_recd(t_recd, dd):
                # [1,512] DRAM row slice -> [128,512] fp16 via K=1 matmul
                stag = rrp.tile([1, RR * 128], F16, tag="rrow")
                nc_.sync.dma_start(
                    out=stag[:],
                    in_=t_recd[:, dd * RR * 128:(dd + 1) * RR * 128])
                ps = pcp.tile([128, RR * 128], F32, tag="bc", space="PSUM")
                nc_.tensor.matmul(out=ps[:], lhsT=ones1[:1, :],
                                  rhs=stag[:1, :], start=True, stop=True)
                recd_dd = rp.tile([128, RR * 128], F16, tag="recd")
                nc_.scalar.activation(
                    out=recd_dd[:], in_=ps[:],
                    func=mybir.ActivationFunctionType.Copy)
                return recd_dd

            def consume_group(st, g, msgs, is_sb, t_recd, FF, w_sb,
                              b_sb, HH, out_dram, relu, out_pad=0):
                items = st.items[g]
                sbatch = {}
                for b0 in range(0, len(items), SB):
                    bk = min(SB, len(items) - b0)
                    i0 = items[b0][6]
                    assert all(items[b0 + j][6] == i0 + j
                               for j in range(bk))
                    S = sp.tile([128, SB * 128], F16, tag="S")
                    bis = is_sb[:, i0:i0 + bk].unsqueeze(2).broadcast_to(
                        (128, bk, 128))
                    sv = S[:, :bk * 128].rearrange("p (k e) -> p k e",
                                                   e=128)
                    nc_.vector.tensor_tensor(
                        out=sv, in0=bis,
                        in1=cseg[:].unsqueeze(1).broadcast_to(
                            (128, bk, 128)),
                        op=mybir.AluOpType.is_equal)
                    for j in range(bk):
                        sbatch[b0 + j] = (S, j)
                psumA = {}
                for ii, (go, t, dd, r, first, last, it) in enumerate(items):
                    if dd not in psumA:
                        pa_tile = pap.tile([128, RR * 128], F32,
                                           tag="A", space="PSUM")
                        psumA[dd] = pa_tile
                    S, j = sbatch[ii]
                    nc_.tensor.matmul(
                        out=psumA[dd][:, r * 128:(r + 1) * 128],
                        lhsT=msgs[go][:, t * FF:(t + 1) * FF],
                        rhs=S[:, j * 128:(j + 1) * 128],
                        start=bool(first), stop=bool(last))
                for dd in sorted(psumA.keys()):
                    recd_dd = make_recd(t_recd, dd)
                    aggT = agp.tile([128, RR * 128], F16, tag="aggT")
                    nc_.scalar.activation(
                        out=aggT[:], in_=psumA[dd][:],
                        func=mybir.ActivationFunctionType.Copy)
                    nc_.vector.tensor_tensor(
                        out=aggT[:], in0=aggT[:], in1=recd_dd[:],
                        op=mybir.AluOpType.mult)
                    psumB = pbp.tile([128, HH], F32, tag="B",
                                     space="PSUM")
                    for r in range(RR):
                        is_last = (r == RR - 1) and (b_sb is None)
                        nc_.tensor.matmul(
                            out=psumB[:],
                            lhsT=aggT[:, r * 128:(r + 1) * 128],
                            rhs=w_sb[:, r * HH:(r + 1) * HH],
                            start=(r == 0), stop=is_last)
                    if b_sb is not None:
                        nc_.tensor.matmul(
                            out=psumB[:], lhsT=ones1[:1, :],
                            rhs=b_sb[:1, :], start=False, stop=True)
                    o_sb = wp.tile([128, HH + out_pad], F16, tag="osb")
                    if out_pad:
                        nc_.vector.memset(o_sb[:, HH:], 0.0)
                    nc_.scalar.activation(
                        out=o_sb[:, :HH], in_=psumB[:],
                        func=(mybir.ActivationFunctionType.Relu if relu
                              else mybir.ActivationFunctionType.Copy))
                    nc_.sync.dma_start(
                        out=out_dram[dd * 128:(dd + 1) * 128, :],
                        in_=o_sb[:])

            # L2 gather calls grouped by src chunk (one chunk == one L1
            # output group) so they can be emitted as h1 tiles land.
            l2_calls_by_chunk = {}
            for g in range(st2.NG):
                for call in st2.calls[g]:
                    l2_calls_by_chunk.setdefault(call[1], []).append(call)
            msgs2 = {}

            def emit_l2_chunk(m):
                for (go, mm, col0, L, ntl) in l2_calls_by_chunk.get(m, []):
                    msg = msg2p.tile([128, ntl * H], F16, tag="msg2")
                    if L % 128:
                        nc_.vector.memset(msg[:, (ntl - 1) * H:], 0.0)
                    lo = mm * st2.chs
                    hi = min(NDP1, lo + st2.chs)
                    nc_.gpsimd.dma_gather(
                        out_ap=msg[:].rearrange("p (c e) -> p c e", e=H),
                        in_ap=h1_part[lo:hi, :],
                        idxs_ap=idx2_sb[:, col0:col0 + (L + 15) // 16],
                        num_idxs=L, num_idxs_reg=L, elem_size=H)
                    msgs2[go] = msg

            # ---- layer 1 ----
            LAG = 2
            for g in range(st1.NG):
                gc0, gc1 = st1.gcols[g]
                idx_sb = gp_idx.tile([128, gc1 - gc0], I16, tag="gidx")
                nc_.sync.dma_start(out=idx_sb[:], in_=t_idx1[:, gc0:gc1])
                msgs = {}
                for (go, mm, col0, L, ntl) in st1.calls[g]:
                    msg = msgp.tile([128, ntl * F], F16, tag="msg")
                    if L % 128:
                        nc_.vector.memset(msg[:, (ntl - 1) * F:], 0.0)
                    lo = mm * st1.chs
                    hi = min(dims['N'], lo + st1.chs)
                    nc_.gpsimd.dma_gather(
                        out_ap=msg[:].rearrange("p (c e) -> p c e", e=F),
                        in_ap=t_x16[lo:hi, :],
                        idxs_ap=idx_sb[:, col0 - gc0:
                                       col0 - gc0 + (L + 15) // 16],
                        num_idxs=L, num_idxs_reg=L, elem_size=F)
                    msgs[go] = msg
                consume_group(st1, g, msgs, is1_sb, t_recd1, F,
                              w1_sb, b1_sb, H, h1_part, relu=True)
                if g - LAG >= 0:
                    emit_l2_chunk(g - LAG)
            for m in range(max(0, st1.NG - LAG), dims['CH2']):
                emit_l2_chunk(m)

            # ---- layer 2 (partial aggregates over ALL T, local h1) ----
            for g in range(st2.NG):
                consume_group(st2, g, msgs2, is2_sb, t_recd2, H,
                              w2_sb, None, EMB, nd_part, relu=False,
                              out_pad=128 - EMB)

            nc_.gpsimd.collective_compute(
                "AllReduce", mybir.AluOpType.add,
                replica_groups=[list(range(NC))],
                ins=[nd_part[:]], outs=[nd_full[:]])

            # ---- final pair MLP (replicated on every core) ----
            nfT = cpool.tile([128, B], F16)
            ffT = cpool.tile([128, B], F16)
            HB = B // 2
            for h in range(2):
                nc_.gpsimd.dma_gather(
                    out_ap=nfT[:, h * HB:(h + 1) * HB].rearrange(
                        "p (c e) -> p c e", c=1),
                    in_ap=nd_full[:, :],
                    idxs_ap=nest_sb[:, h * (HB // 16):(h + 1) * (HB // 16)],
                    num_idxs=HB, num_idxs_reg=HB, elem_size=128,
                    transpose=True)
                nc_.gpsimd.dma_gather(
                    out_ap=ffT[:, h * HB:(h + 1) * HB].rearrange(
                        "p (c e) -> p c e", c=1),
                    in_ap=nd_full[:, :],
                    idxs_ap=food_sb[:, h * (HB // 16):(h + 1) * (HB // 16)],
                    num_idxs=HB, num_idxs_reg=HB, elem_size=128,
                    transpose=True)
            for pt in range(B // 128):
                psumD = pbp.tile([128, H], F32, tag="B", space="PSUM")
                nc_.tensor.matmul(out=psumD[:],
                                  lhsT=nfT[:, pt * 128:(pt + 1) * 128],
                                  rhs=fca_sb[:], start=True, stop=False)
                nc_.tensor.matmul(out=psumD[:],
                                  lhsT=ffT[:, pt * 128:(pt + 1) * 128],
                                  rhs=fcb_sb[:], start=False, stop=False)
                nc_.tensor.matmul(out=psumD[:], lhsT=ones1[:1, :],
                                  rhs=fbias_sb[:1, :], start=False,
                                  stop=True)
                fo = wp.tile([128, H], F32, tag="fo")
                nc_.scalar.activation(out=fo[:], in_=psumD[:],
                                      func=mybir.ActivationFunctionType.Tanh)
                nc_.sync.dma_start(out=t_out[pt * 128:(pt + 1) * 128, :],
                                   in_=fo[:])
    # Spread Pool-engine DMAs across the 4 SWDGE queues.  Each DMASW
    # completion-sem lane must stay locked to one queue (per-queue FIFO
    # completion order); the tile layer already assigned lanes, so read
    # them back and set queue = lane % 4.
    from concourse.tile_scheduler import PROC_NAMES
    for bb in nc_.main_func.blocks:
        for inst in bb.instructions:
            proc = getattr(inst, 'bass_scheduled_proc', None)
            if proc is None:
                continue
            pname = PROC_NAMES[proc]
            if not pname.startswith("DMASW"):
                continue
            q = int(pname[5:]) % NQUEUES
            if isinstance(inst, mybir.InstDMAGatherAnt):
                inst.queue_num = q
            elif isinstance(inst, mybir.InstDMACopy):
                qname = str(getattr(inst, 'queue', '') or '')
                if qname.startswith("qPoolDynamic"):
                    inst.queue = "qPoolDynamic" + (str(q) if q else "")
    nc_.compile()
    return nc_


def kernel(x, edge_src, edge_dst, edge_type, edge_attr, nest, food,
           W_rel1, W_root1, b1, W_rel2, W_root2, b2, fc_W, fc_b,
           _runner=None):
    x = np.asarray(x, np.float32)
    N, F = x.shape
    H = np.asarray(W_root1).shape[1]
    EMB = np.asarray(W_root2).shape[1]
    inputs = dict(edge_src=edge_src, edge_dst=edge_dst,
                  edge_type=edge_type, nest=nest, food=food, x=x,
                  W_rel1=W_rel1)
    st1, st2, dims, nestw, foodw, recd1, recd2 = preprocess(inputs)
    nc_ = build(st1, st2, dims, F, H, EMB)

    RR = dims['RR']
    W_rel1 = np.asarray(W_rel1, np.float32)
    W_rel2 = np.asarray(W_rel2, np.float32)
    w1 = np.concatenate([W_rel1.transpose(1, 0, 2).reshape(F, -1),
                         np.asarray(W_root1, np.float32)],
                        axis=1).astype(np.float16)
    w2 = np.concatenate([W_rel2.transpose(1, 0, 2).reshape(H, -1),
                         np.asarray(W_root2, np.float32)],
                        axis=1).astype(np.float16)
    fc_W = np.asarray(fc_W, np.float32)
    fca = np.zeros((128, H), np.float32)
    fca[:EMB] = fc_W[:EMB]
    fcb = np.zeros((128, H), np.float32)
    fcb[:EMB] = fc_W[EMB:]
    b2f = np.asarray(b2, np.float32)
    fbias = (np.asarray(fc_b, np.float32) +
             b2f @ (fc_W[:EMB] + fc_W[EMB:])).reshape(1, -1)
    x16 = x.astype(np.float16)

    in_maps = []
    for c in range(NC):
        in_maps.append(dict(
            x16=x16,
            idx1=st1.idx128[c], is1=st1.iseg[c],
            recd1=recd1[c:c + 1],
            idx2=st2.idx128[c], is2=st2.iseg[c], recd2=recd2,
            w1=w1, b1=np.asarray(b1, np.float16).reshape(1, -1),
            w2=w2,
            fca=fca.astype(np.float16), fcb=fcb.astype(np.float16),
            fbias=fbias.astype(np.float16),
            nestw=nestw, foodw=foodw,
        ))
    runner = _runner or (lambda n, im: run_bass_kernel_spmd(
        n, im, core_ids=list(range(NC))))
    res = runner(nc_, in_maps)
    return res.results[0]["out"]
